# revision 130
# baseline (speedup 1.0000x reference)
"""Multi-head attention (B=2, S=2048, D=1024, H=16) on 8 TRN2 NeuronCores.

Sharding: data-parallel over batch (2 groups of 4 cores) x head-parallel
(4 heads per core). W_q/W_k/W_v are column-sharded by head, W_o is
row-sharded; the 4 partial W_o outputs per batch are summed on the host
(the unshard step), which also undoes the device-side transposed layout.

Per-core kernel v3 (swapped-PV rewrite of v2, 169.8us -> 125.0us sim):
  - All matmul operands are bf16 (qT/kT/Wo dropped from f32r): immune to
    the fp32r mid-pstate row penalty and halves their SBUF/DMA footprint.
  - P@V runs in swapped orientation: probs [k, q] is the stationary
    operand, v [k, 65] the moving one, so the output is [128 q, 65] --
    full 128-partition PE array use (65-row instructions) instead of the
    v2 [65, 256] layout that idled half the array (73728 -> ~35k rows).
    The four (hh, c) accumulators share one PSUM bank, so they run as
    SEQUENTIAL passes over a tile-group's collected probs tiles (the
    2KB PSUM zero-region admits only one open accumulation group).
  - The softmax denominator rides along as v's 65th (ones) column; with
    q on partitions the normalize is a native per-partition reciprocal +
    tensor_scalar multiply on DVE (no gpsimd broadcast). gpsimd cannot
    read PSUM on this target -- all PSUM->SBUF copies live on DVE.
  - [q, d] -> [d, q] for the output projection via PE matmul-transpose
    (128 rows each, ~2us total, host-supplied identity): the single-FIFO
    DMA engine would stall crossbar transposes behind bulk loads, and
    per-engine FIFO DMA-completion counters would false-order them.
  - A diagonal causal block with base -128 is fully masked in its low
    128 q columns: only the live half is computed and the pair's exp
    packs to 384 wide (activation is the steady-state governor at
    1038ns/pair; 72 exps = 72.6us busy).
  - Software-pipelined emission: scores/exp lead, P@V passes lag one
    tile-group, and a ready-ticked work queue spreads outproj/late-
    projection/v-projection units into the exp-governed bubbles (PE
    in-order queues can only absorb waits with work emitted BEFORE the
    waiting instruction). Tile order [1..6, 0, 7] + just-in-time DMA
    ordering keep the serialized DMA stream off the critical path.
  - PSUM: scores 2x2-bank tiles, PV accumulators [128,4,65] 1 bank x2,
    shared proj/outproj/transpose pool [128,512] x2 -- exactly 8 banks.
"""

import os

import numpy as np

_B, _S, _D, _H, _DK = 2, 2048, 1024, 16, 64
_HPC = 4          # heads per core
_NCORES = 8
_CPG = 4          # cores per (batch) group
_DPC = _HPC * _DK # 256 projection dims per core
_NEG = -1e9
_SQW = 256        # sq tile width for attention
_NSQ = _S // _SQW # 8 sq tiles

_program_cache = {}
LAST_RESULTS = None  # BassKernelResults of the most recent run (for profiling)


def _analyze_mask(mask):
    """Classify each [128 k, 256 sq] block of mask^T. Returns (plan, dense).

    plan[i] = tuple of (j, mode, param) for sq-tile i; mode 0 = no mask,
    1 = causal affine_select (param = base), 2 = dense additive mask
    (param = index into dense blocks). Fully-masked blocks are omitted.
    Within a tile, masked (mode!=0) blocks come first so their extra
    post-exp work overlaps the remaining blocks' matmuls.
    """
    maskT = np.ascontiguousarray(mask.T)
    plan = []
    dense = []
    p_idx = np.arange(128)[:, None]
    s_idx = np.arange(_SQW)[None, :]
    for i in range(_NSQ):
        row = []
        for j in range(_S // 128):
            blk = maskT[j * 128:(j + 1) * 128, i * _SQW:(i + 1) * _SQW]
            nz = blk != 0.0
            if nz.all():
                continue  # fully masked: block contributes nothing
            if not nz.any():
                row.append((j, 0, 0))
                continue
            base = i * _SQW - j * 128
            causal = (s_idx + i * _SQW) < (p_idx + j * 128)
            if np.array_equal(nz, causal) and np.all(blk[nz] == 1.0):
                row.append((j, 1, base))
            else:
                row.append((j, 2, len(dense)))
                dense.append(blk * np.float32(_NEG))
        row.sort(key=lambda b: b[1] == 0)  # masked blocks first
        plan.append(tuple(row))
    if dense:
        dense_np = np.stack(dense).astype(np.float32)
    else:
        dense_np = np.zeros((1, 128, _SQW), np.float32)
    return tuple(plan), dense_np


def _build_program(plan, nblk):
    import concourse.bass as bass  # noqa: F401  (registers engine classes)
    import concourse.tile as tile
    from concourse import bacc, mybir

    F32 = mybir.dt.float32
    BF16 = mybir.dt.bfloat16
    AF = mybir.ActivationFunctionType
    ALU = mybir.AluOpType
    ts = bass.ts

    nc = bacc.Bacc(None, target_bir_lowering=False, debug=False)

    xq = nc.dram_tensor("xq", [_D, _S], BF16, kind="ExternalInput").ap()
    xk = nc.dram_tensor("xk", [_D, _S], BF16, kind="ExternalInput").ap()
    xv = nc.dram_tensor("xv", [_D, _S], BF16, kind="ExternalInput").ap()
    wq = nc.dram_tensor("wq", [_D, _DPC], BF16, kind="ExternalInput").ap()
    wk = nc.dram_tensor("wk", [_D, _DPC], BF16, kind="ExternalInput").ap()
    wv = nc.dram_tensor("wv", [_D, _DPC], BF16, kind="ExternalInput").ap()
    wo = nc.dram_tensor("wo", [_DPC, _D], BF16, kind="ExternalInput").ap()
    bq = nc.dram_tensor("bq", [_DPC], F32, kind="ExternalInput").ap()
    bk = nc.dram_tensor("bk", [_DPC], F32, kind="ExternalInput").ap()
    ident = nc.dram_tensor("ident", [128, 128], BF16, kind="ExternalInput").ap()
    mblk = nc.dram_tensor("mblk", [nblk, 128, _SQW], F32,
                          kind="ExternalInput").ap()
    y = nc.dram_tensor("y", [_D, _S], BF16, kind="ExternalOutput").ap()

    has_dense = any(m == 2 for row in plan for (_, m, _) in row)
    resident_mask = has_dense and nblk <= 8
    need_stream = has_dense and not resident_mask

    with tile.TileContext(nc) as tc:
        from contextlib import ExitStack
        with ExitStack() as ctx:
            wpool = ctx.enter_context(tc.tile_pool(name="w", bufs=1))
            cpool = ctx.enter_context(tc.tile_pool(name="const", bufs=1))
            xpool = ctx.enter_context(tc.tile_pool(name="xres", bufs=1))
            biga = ctx.enter_context(tc.tile_pool(name="biga", bufs=1))
            probp = ctx.enter_context(tc.tile_pool(name="probs", bufs=16))
            recp = ctx.enter_context(tc.tile_pool(name="rec", bufs=4))
            aswp = ctx.enter_context(tc.tile_pool(name="asw", bufs=4))
            yp = ctx.enter_context(tc.tile_pool(name="y", bufs=3))
            mpool = (
                ctx.enter_context(tc.tile_pool(name="mstream", bufs=3))
                if need_stream else None
            )
            spsp = ctx.enter_context(tc.tile_pool(name="sps", bufs=2,
                                                  space="PSUM"))
            accp = ctx.enter_context(tc.tile_pool(name="acc", bufs=2,
                                                  space="PSUM"))
            gp = ctx.enter_context(tc.tile_pool(name="gp", bufs=2,
                                                space="PSUM"))

            xq_r = xq.rearrange("(m p) s -> p m s", p=128)
            xk_r = xk.rearrange("(m p) s -> p m s", p=128)
            xv_r = xv.rearrange("(m p) s -> p m s", p=128)
            wq_r = wq.rearrange("(m p) d -> p m d", p=128)
            wk_r = wk.rearrange("(m p) d -> p m d", p=128)
            wv_r = wv.rearrange("(m p) d -> p m d", p=128)

            # resident SBUF state
            xq_sb = xpool.tile([128, 8, _S], BF16, tag="xq")
            xk_sb = xpool.tile([128, 8, _S], BF16, tag="xk")
            xv_sb = xpool.tile([128, 8, _S], BF16, tag="xv")
            wq_sb = wpool.tile([128, 8, _DPC], BF16, tag="wq")
            wk_sb = wpool.tile([128, 8, _DPC], BF16, tag="wk")
            wv_sb = wpool.tile([128, 8, _DPC], BF16, tag="wv")
            wo_sb = wpool.tile([128, 2, _D], BF16, tag="wo")
            bq_sb = cpool.tile([128, 2], F32, tag="bq")
            bk_sb = cpool.tile([128, 2], F32, tag="bk")

            # --- DMA emission: 2-m-chunk interleave of the st0 q/k loads so
            # the first projection matmuls start ~1us in and stream behind
            # the arrivals
            for lo in (0, 2, 4, 6):
                nc.sync.dma_start(out=wq_sb[:, lo:lo + 2],
                                  in_=wq_r[:, lo:lo + 2])
                nc.sync.dma_start(out=xq_sb[:, lo:lo + 2, 0:512],
                                  in_=xq_r[:, lo:lo + 2, 0:512])
                if lo == 0:
                    nc.sync.dma_start(out=bq_sb,
                                      in_=bq.rearrange("(h p) -> p h", p=128))
                    nc.sync.dma_start(out=bk_sb,
                                      in_=bk.rearrange("(h p) -> p h", p=128))
            ident_sb = cpool.tile([128, 128], BF16, tag="ident")
            nc.sync.dma_start(out=ident_sb, in_=ident)
            for lo in (0, 2, 4, 6):
                nc.sync.dma_start(out=wk_sb[:, lo:lo + 2],
                                  in_=wk_r[:, lo:lo + 2])
                nc.sync.dma_start(out=xk_sb[:, lo:lo + 2, 0:512],
                                  in_=xk_r[:, lo:lo + 2, 0:512])
            # just-in-time order on the single serialized DMA stream for
            # tile order [1..7, 0]: each 512-column group of q/k/v arrives
            # shortly before its consumer tiles; wo before the first outproj
            nc.sync.dma_start(out=wv_sb, in_=wv_r)
            nc.sync.dma_start(out=xv_sb[:, :, 0:512], in_=xv_r[:, :, 0:512])
            nc.sync.dma_start(out=xq_sb[:, :, ts(1, 512)], in_=xq_r[:, :, ts(1, 512)])
            nc.sync.dma_start(out=xk_sb[:, :, ts(1, 512)], in_=xk_r[:, :, ts(1, 512)])
            nc.sync.dma_start(out=wo_sb, in_=wo.rearrange("(c p) o -> p c o", p=128))
            nc.sync.dma_start(out=xv_sb[:, :, ts(1, 512)], in_=xv_r[:, :, ts(1, 512)])
            for st in (2, 3):
                nc.sync.dma_start(out=xq_sb[:, :, ts(st, 512)],
                                  in_=xq_r[:, :, ts(st, 512)])
                nc.sync.dma_start(out=xk_sb[:, :, ts(st, 512)],
                                  in_=xk_r[:, :, ts(st, 512)])
                nc.sync.dma_start(out=xv_sb[:, :, ts(st, 512)],
                                  in_=xv_r[:, :, ts(st, 512)])
            if resident_mask:
                mask_sb = cpool.tile([128, nblk, _SQW], F32, tag="mask")
                nc.sync.dma_start(out=mask_sb,
                                  in_=mblk.rearrange("n p s -> p n s"))

            # --- big SBUF state ---
            qT = biga.tile([128, 2, _S], BF16, tag="qT")
            kT = biga.tile([128, 2, _S], BF16, tag="kT")
            vsb = biga.tile([128, 16, _HPC * 65], BF16, tag="v")
            attnT = biga.tile([128, 2, _S], BF16, tag="attnT")

            # ones columns of v (softmax denominator trick): one strided memset
            nc.vector.memset(
                vsb.rearrange("p a (h x) -> p a h x", x=65)[:, :, :, 64:65], 1.0
            )

            # v-projection per 512-wide k-column group, in two half units
            v_units = {}  # col -> list of not-yet-emitted half closures

            def _v_half(col, half):
                s0 = col * 4 + half * 2
                vps = gp.tile([128, 512], F32, tag="mm", name="vps")
                for sub in range(2):
                    sc = s0 + sub
                    for m in range(8):
                        nc.tensor.matmul(
                            vps[:, ts(sub, _DPC)],
                            lhsT=xv_sb[:, m, ts(sc, 128)],
                            rhs=wv_sb[:, m, :],
                            start=(m == 0), stop=(m == 7),
                        )
                # on DVE: the Pool queue head-of-line-blocks behind
                # affine_selects that wait on future exps
                nc.vector.tensor_copy(
                    vsb[:, s0:s0 + 2, 0:260].rearrange(
                        "p a (h x) -> p a h x", x=65)[:, :, :, 0:64],
                    vps.rearrange("p (a h x) -> p a h x", a=2, x=64),
                )

            v_emitted = set()

            def _v_half_once(col, half):
                if (col, half) in v_emitted:
                    return
                v_emitted.add((col, half))
                _v_half(col, half)

            def ensure_vgroup(col):
                # inline fallback: halves not yet spread via the work queue
                # must be emitted before the consumer P@V matmuls
                _v_half_once(col, 0)
                _v_half_once(col, 1)

            def queue_vgroup(col, delay):
                # spread the two 853ns halves through attention pairs
                # instead of bursting them inside a P@V pass (popped-late
                # duplicates are no-ops)
                for h in range(2):
                    work_queue.append((tick[0] + delay, "pe",
                                       lambda c=col, h=h: _v_half_once(c, h)))


            def proj_unit(st, dh, which, half, width):
                # one q/k projection chunk (width cols of column-group st)
                x_sb, w_sb, b_sb, out = (
                    (xq_sb, wq_sb, bq_sb, qT) if which == 0
                    else (xk_sb, wk_sb, bk_sb, kT))
                c0 = st * 512 + half * width
                pps = gp.tile([128, 512], F32, tag="mm", name="pps")
                for m in range(8):
                    nc.tensor.matmul(
                        pps[:, 0:width], lhsT=w_sb[:, m, ts(dh, 128)],
                        rhs=x_sb[:, m, c0:c0 + width],
                        start=(m == 0), stop=(m == 7),
                    )
                nc.vector.tensor_scalar(
                    out[:, dh, c0:c0 + width], pps[:, 0:width],
                    b_sb[:, dh:dh + 1], None, ALU.add,
                )

            def stageA(i, g, pair):
                # scores + exp for a pair of blocks; returns (probs, layout).
                # A mode-1 block with base == -128 has its first 128 q
                # columns fully masked: only its live right half is computed
                # and the pair is packed tight so one 384-wide exp covers it.
                layout = []
                off = 0
                for (j, mode, param) in pair:
                    w = 128 if (mode == 1 and param == -128) else _SQW
                    layout.append((j, mode, param, off, w))
                    off += w
                width = off
                sps = spsp.tile([128, 2, 512], F32, tag="sps", name="sps")
                for (j, mode, param, off, w) in layout:
                    qoff = i * _SQW + (_SQW - w)
                    for hh in range(2):
                        # the two head slots sit in separate psum banks:
                        # matmuls into one bank must share a tile position
                        # (device constraint) and hh=0/hh=1 have base
                        # partitions 0/64
                        nc.tensor.matmul(
                            sps[:, hh, off:off + w],
                            lhsT=kT[hh * 64:(hh + 1) * 64, g, ts(j, 128)],
                            rhs=qT[hh * 64:(hh + 1) * 64, g, qoff:qoff + w],
                            start=True, stop=True,
                        )
                    if mode == 2:
                        if resident_mask:
                            mt = mask_sb[:, param, :]
                        else:
                            mt = mpool.tile([128, _SQW], F32, tag="mtile",
                                            name="mt")
                            nc.sync.dma_start(out=mt, in_=mblk[param])
                        for hh in range(2):
                            nc.vector.tensor_add(sps[:, hh, off:off + w],
                                                 sps[:, hh, off:off + w], mt)
                probs = probp.tile([128, 2, 512], BF16, tag="probs",
                                   name="probs")
                nc.scalar.activation(probs[:, :, 0:width], sps[:, :, 0:width],
                                     AF.Exp)
                for (j, mode, param, off, w) in layout:
                    if mode != 1:
                        continue
                    # masked cells satisfy s < p - base, p <= 127; for the
                    # packed half-block the condition reduces to s' < p
                    base = 0 if w == 128 else param
                    ncols = min(w, 128 - base)
                    if ncols > 0:
                        nc.gpsimd.affine_select(
                            out=probs[:, :, off:off + ncols],
                            in_=probs[:, :, off:off + ncols],
                            compare_op=ALU.is_ge, fill=0.0,
                            base=base, channel_multiplier=-1,
                            pattern=[[0, 2], [1, ncols]],
                        )
                return probs, layout

            def pv_pass(grp, pi_):
                # swapped P@V: probs [k, q] stationary, v [k, 65] moving;
                # acc [128 q, (c, hh), 65]. The PSUM zero-region (one bank)
                # admits only ONE open accumulation group, so the four
                # (hh, c) groups run as SEQUENTIAL passes over the group's
                # collected probs tiles. Packed half-blocks (w=128) only
                # contribute to the c=1 half.
                i, g = grp["i"], grp["g"]
                hh, c = pi_ // 2, pi_ % 2
                if pi_ == 0:
                    grp["acc"] = accp.tile([128, 4, 65], F32, tag="acc",
                                           name=f"acc{i}_{g}")
                acc = grp["acc"]
                hv = 2 * g + hh
                gi = c * 2 + hh
                items = [
                    (layout_t, probs)
                    for (layout, probs) in grp["plist"]
                    for layout_t in layout
                    if not (layout_t[4] == 128 and c == 0)
                ]
                for bi, ((j, mode, param, off, w), probs) in enumerate(items):
                    ensure_vgroup(j // 4)
                    po = off if w == 128 else off + c * 128
                    nc.tensor.matmul(
                        acc[:, gi, :],
                        lhsT=probs[:, hh, po:po + 128],
                        rhs=vsb[:, j, hv * 65:(hv + 1) * 65],
                        start=(bi == 0),
                        stop=(bi == len(items) - 1),
                    )
                if pi_ == 3:
                    finish_group(grp)

            def finish_group(grp):
                # normalize: acc col 64 holds the denominators, already
                # per-partition (q) -> native tensor_scalar broadcast
                i, g, acc = grp["i"], grp["g"], grp["acc"]
                rec_t = recp.tile([128, 4, 1], F32, tag="rec", name="rec")
                with nc.allow_low_precision(reason="softmax reciprocal"):
                    nc.vector.reciprocal(rec_t, acc[:, :, 64:65])
                asw = aswp.tile([128, 2, 128], BF16, tag="asw", name="asw")
                for c in range(2):
                    for hh in range(2):
                        nc.vector.tensor_scalar(
                            asw[:, c, hh * 64:(hh + 1) * 64],
                            acc[:, c * 2 + hh, 0:64],
                            rec_t[:, c * 2 + hh, :], None, ALU.mult,
                        )
                # [q, d] -> [d, q]: PE matmul-transpose (128 rows each,
                # ~2us total) + DVE copy to SBUF. Deferred one pair so the
                # transpose's wait on the DVE normalize chain never sits in
                # front of the next pair's scores in the in-order PE queue.
                def do_transpose(i=i, g=g, asw=asw):
                    tr = gp.tile([128, 2, 128], BF16, tag="mm", name="tr")
                    for c in range(2):
                        nc.tensor.transpose(tr[:, c, :], asw[:, c, :],
                                            ident_sb)
                        nc.vector.tensor_copy(
                            attnT[:, g,
                                  i * _SQW + c * 128:i * _SQW + (c + 1) * 128],
                            tr[:, c, :],
                        )
                work_queue.append((tick[0] + 1, "pe", do_transpose))
                if g == 1:
                    s = seq_pos[i]
                    queue_outproj(i, s % 2, tile_seq[s - 1] if s > 0 else -9)

            ysb_box = [None]
            tail_mode = [False]
            work_queue = None  # set below (deque of outproj closures)

            def outproj_unit(i, half, oc):
                # one output-projection 2-oc chunk for 256-wide sq tile i;
                # emitted piecemeal between attention pairs so the PE filler
                # spreads across the exp-governed stretches
                if half == 0 and oc == 0:
                    ysb_box[0] = yp.tile([128, 8, 512], BF16, tag="y",
                                         name="ysb")
                ysb = ysb_box[0]
                yps = gp.tile([128, 512], F32, tag="mm", name="yps")
                for sub in range(2):
                    for cc in range(2):
                        nc.tensor.matmul(
                            yps[:, ts(sub, _SQW)],
                            lhsT=wo_sb[:, cc, ts(oc + sub, 128)],
                            rhs=attnT[:, cc, ts(i, _SQW)],
                            start=(cc == 0), stop=(cc == 1),
                        )
                # always DVE: gpsimd cannot read PSUM on this target
                nc.vector.tensor_copy(
                    ysb[:, oc:oc + 2, half * _SQW:(half + 1) * _SQW],
                    yps.rearrange("p (a x) -> p a x", a=2),
                )

            def ydma_unit(i, prev, oc):
                ysb = ysb_box[0]
                r = slice(oc * 128, (oc + 1) * 128)
                nc.sync.dma_start(out=y[r, prev * _SQW:(i + 1) * _SQW],
                                  in_=ysb[:, oc, :])

            def ydma_single(i, half, oc):
                ysb = ysb_box[0]
                r = slice(oc * 128, (oc + 1) * 128)
                # in the drain the 650ns-per-issue HWDGE chain IS the tail:
                # route 3 of the last 8 stores through the Pool SWDGE path
                # (separate DMA-generation device, Pool idle by then)
                eng = nc.gpsimd if (tail_mode[0] and oc % 3 == 1) else nc.sync
                eng.dma_start(out=y[r, ts(i, _SQW)],
                              in_=ysb[:, oc, half * _SQW:(half + 1) * _SQW])

            def queue_outproj(i, half, prev):
                # ready >= tick+2: attnT(i) finishes ~1-2 pairs after the
                # queueing flush (normalize -> transpose -> copy chain), and
                # a queued unit that isn't ready would block the in-order PE
                # queue instead of filling it. Also back-loaded (ready >=
                # 26 + 5*seq): the PE bubbles all sit in the late big-tile
                # stretches where the exp engine governs the pace, so early
                # tiles' outproj makes better filler there.
                s = seq_pos[i]
                ready = max(tick[0] + 2, 26 + 5 * s)
                nxt = tile_seq[s + 1] if s + 1 < len(tile_seq) else None
                split = (half == 0 and (nxt is None or abs(i - nxt) != 1)) \
                    or (half == 1 and (i == 0 or prev == 0
                                       or abs(i - prev) != 1))
                # each 2-oc outproj unit is immediately followed by its own
                # store issues: in the drain, the HWDGE issue (650ns, a
                # separate device) then overlaps the next unit's matmul+copy
                # chain instead of serializing after ALL copies
                for oc in range(0, 8, 2):
                    work_queue.append((ready, "pe",
                                       lambda i=i, h=half, oc=oc:
                                       outproj_unit(i, h, oc)))
                    if split:
                        # non-adjacent pair: per-tile stores, fired as soon
                        # as this tile's own copies land
                        for o2 in (oc, oc + 1):
                            work_queue.append((ready + 1, "free",
                                               lambda i=i, h=half, o2=o2:
                                               ydma_single(i, h, o2)))
                    elif half == 1:
                        for o2 in (oc, oc + 1):
                            work_queue.append((ready + 1, "free",
                                               lambda i=i, p=prev, o2=o2:
                                               ydma_unit(i, p, o2)))

            # two-pair-lookahead software pipeline: scores+exp run two pairs
            # ahead of P@V so the exp latency (the attention-phase governor)
            # never gates consecutive PE instructions. Tile 0 (one pair of
            # half-masked blocks) runs LAST: it needs only st0 data and
            # makes the serial tail as small as possible.
            from collections import deque
            group_q = deque()
            work_queue = deque()
            tick = [0]
            tile_seq = [1, 2, 3, 4, 5, 6, 0, 7]
            seq_pos = {i: s for s, i in enumerate(tile_seq)}
            proj_remaining = {}

            def do_pv(n):
                for _ in range(n):
                    if not group_q:
                        return
                    grp = group_q[0]
                    pv_pass(grp, grp["pi"])
                    grp["pi"] += 1
                    if grp["pi"] == 4:
                        group_q.popleft()

            def pop_work(n, force=False):
                # pops up to n PE-work units; 'free' units (DMA issues with
                # no PE instructions) don't count toward the budget. Scans
                # past not-yet-ready entries (units are independent; the
                # tile framework tracks their data deps).
                while n > 0 and work_queue:
                    idx = None
                    for k in range(len(work_queue)):
                        if force or work_queue[k][0] <= tick[0]:
                            idx = k
                            break
                    if idx is None:
                        return
                    ready, kind, fn = work_queue[idx]
                    del work_queue[idx]
                    fn()
                    if kind == "pe":
                        n -= 1

            def queue_proj(st, delay):
                # late projections spread as fine 128-col units (427ns,
                # matching the per-pair PE bubble of the exp-governed
                # stretches); q units lead k in the queue, matching the
                # x-arrival order
                proj_remaining[st] = 16
                for which in range(2):
                    for dh in range(2):
                        for half in range(4):
                            def unit(st=st, dh=dh, w=which, h=half):
                                proj_unit(st, dh, w, h, 128)
                                proj_remaining[st] -= 1
                            work_queue.append((tick[0] + delay, "pe", unit))

            def drain_proj(st):
                while proj_remaining.get(st, 0) > 0:
                    pop_work(1, force=True)

            def emit_proj(st):
                for dh in range(2):
                    proj_unit(st, dh, 0, 0, 512)
                    proj_unit(st, dh, 1, 0, 512)

            def attention(i):
                blocks = plan[i]
                nj = len(blocks)
                for g in range(2):
                    grp = {"i": i, "g": g, "plist": [], "pi": 0}
                    for pi in range(0, nj, 2):
                        tick[0] += 1
                        probs, layout = stageA(i, g, blocks[pi:pi + 2])
                        grp["plist"].append((layout, probs))
                        # P@V passes of the PREVIOUS group, then filler --
                        # both AFTER the pair's scores: the activation
                        # engine is the steady-state governor, so nothing
                        # may delay the scores feeding the next exp
                        do_pv(1)
                        pop_work(1)
                    group_q.append(grp)

            for st in range(4):
                if st <= 1:
                    emit_proj(st)  # burst: still inside the DMA-paced ramp
                else:
                    drain_proj(st)
                tiles = [x for x in (2 * st, 2 * st + 1) if x != 0]
                if st == 3:
                    tiles = [6, 0, 7]
                for i in tiles:
                    if i == 2:
                        # st2 spreads over tiles 2-3 (x arrives ~27-30us),
                        # st3 over tiles 4-5; barriers at the st loop heads.
                        # v halves spread ahead of their first P@V pass.
                        queue_vgroup(1, 2)
                        queue_proj(2, 5)
                    elif i == 4:
                        queue_vgroup(2, 1)
                        queue_proj(3, 1)
                    elif i == 5:
                        queue_vgroup(3, 1)
                    attention(i)
            tail_mode[0] = True
            while group_q:
                tick[0] += 1
                do_pv(1)
                pop_work(2, force=True)
            while work_queue:
                pop_work(1, force=True)

    nc.compile()
    return nc


def kernel(**inputs):
    global LAST_RESULTS
    from concourse.bass_utils import run_bass_kernel_spmd

    Q = np.asarray(inputs["Q"], dtype=np.float32)
    K = np.asarray(inputs["K"], dtype=np.float32)
    V = np.asarray(inputs["V"], dtype=np.float32)
    mask = np.asarray(inputs["mask"], dtype=np.float32)
    Wq = np.asarray(inputs["Wq"], dtype=np.float32)
    bq = np.asarray(inputs["bq"], dtype=np.float32)
    Wk = np.asarray(inputs["Wk"], dtype=np.float32)
    bk = np.asarray(inputs["bk"], dtype=np.float32)
    Wv = np.asarray(inputs["Wv"], dtype=np.float32)
    bv = np.asarray(inputs["bv"], dtype=np.float32)
    Wo = np.asarray(inputs["Wo"], dtype=np.float32)
    bo = np.asarray(inputs["bo"], dtype=np.float32)

    plan, dense = _analyze_mask(mask)
    key = (plan, dense.shape[0])
    if key not in _program_cache:
        _program_cache[key] = _build_program(plan, dense.shape[0])
    nc = _program_cache[key]

    import ml_dtypes
    bf16 = ml_dtypes.bfloat16
    sc = np.float32(1.0 / np.sqrt(_DK))
    xqT = [np.ascontiguousarray(Q[b].T).astype(bf16) for b in range(_B)]
    xkT = [np.ascontiguousarray(K[b].T).astype(bf16) for b in range(_B)]
    xvT = [np.ascontiguousarray(V[b].T).astype(bf16) for b in range(_B)]

    in_maps = []
    for core in range(_NCORES):
        b = core // _CPG
        rows = slice((core % _CPG) * _DPC, (core % _CPG) * _DPC + _DPC)
        in_maps.append({
            "xq": xqT[b], "xk": xkT[b], "xv": xvT[b],
            "wq": np.ascontiguousarray((Wq[rows] * sc).T).astype(bf16),
            "wk": np.ascontiguousarray(Wk[rows].T).astype(bf16),
            "wv": np.ascontiguousarray(Wv[rows].T).astype(bf16),
            "wo": np.ascontiguousarray(Wo[:, rows].T).astype(bf16),
            "bq": np.ascontiguousarray(bq[rows] * sc),
            "bk": np.ascontiguousarray(bk[rows]),
            "ident": np.eye(128, dtype=bf16),
            "mblk": dense,
        })

    trace = bool(int(os.environ.get("KERNEL_TRACE", "0")))
    LAST_RESULTS = run_bass_kernel_spmd(
        nc, in_maps, list(range(_NCORES)), trace=trace
    )

    # v-bias folded into the output bias: attn' = attn + bv => y += Wo @ bv
    bo_eff = bo.astype(np.float64) + Wo.astype(np.float64) @ bv.astype(np.float64)
    out = np.empty((_B, _S, _D), np.float32)
    for b in range(_B):
        acc = np.zeros((_D, _S), np.float64)
        for c in range(_CPG):
            acc += np.asarray(LAST_RESULTS.results[b * _CPG + c]["y"],
                              dtype=np.float64)
        out[b] = (acc.T + bo_eff).astype(np.float32)
    return out
